# revision 1
# baseline (speedup 1.0000x reference)
"""DeformConv2d forward on 8 Trainium2 NeuronCores (Bass/Tile).

x[8,128,96,96] f32, offset[8,18,96,96] f32, weight[128,128,3,3] f32
-> out[8,128,96,96] f32. Deformable 3x3 conv, pad 1, stride 1, bilinear
sampling with zero padding. Data-parallel over batch: one element per core.

Per-core pipeline:
  A. x -> fp16, PE-transpose -> x_t[9216,128] in DRAM (pixel-major).
     conv weight -> fp16, PE-transpose -> WkT[ci, k*128+co].
  B. offsets PE-transposed into a position-packed layout [128, 72*18].
  C. DVE index/weight math in a [128, 9*72] packed layout (pos = c*128+p):
     bilinear corner weights A0,A1,B0,B1 (fp16) and pair-row indices
     IDXT, IDXB (top = y0c*96+x0c, bottom = (y1c)*96+x0c, both always
     in-range; out-of-image corners get zero weight).
  D. PE-transpose those to j-ordered DRAM rows (w_rows[36], idx_rows[18]).
  E. idx_rows -> 16-partition-wrapped SBUF layout for dma_gather;
     w_rows -> W36 SBUF.
  F. Per (chunk of 1024 positions, tap): dma_gather pulls (x0,x0+1) fp16
     pixel-pair columns for top and bottom rows (channels on partitions);
     PE broadcasts the 4 slot-weight rows across partitions (ones-matmul
     into PSUM), ACT evacuates to fp16; DVE multiplies gathered pairs by
     slot weights; PE GEMM accumulates over (ci, tap, slot) in PSUM.
"""
import sys
if '/opt/trn_rl_repo' not in sys.path:
    sys.path.insert(0, '/opt/trn_rl_repo')

import numpy as np

import concourse.bacc as bacc_mod
import concourse.mybir as mybir
import concourse.tile as tile
from concourse.ap import AP

f32 = mybir.dt.float32
f16 = mybir.dt.float16
i16 = mybir.dt.int16
i32 = mybir.dt.int32
Alu = mybir.AluOpType

P = 128
H = W = 96
NPOS = H * W              # 9216
NT = NPOS // P            # 72 position tiles
K = 9
NROW = NPOS - 1           # pair windows in x_t
CW = 1536                 # main-loop position chunk
GW = 768                  # per-gather-instruction indices (hw limit <= 896)
NCH = NPOS // CW          # 6 chunks
SUB = 512                 # GEMM moving sub-chunk


def _h(ap_or_handle):
    return ap_or_handle.tensor if hasattr(ap_or_handle, 'tensor') else ap_or_handle


import os
STAGE = os.environ.get("KSTAGE", "F")


def build_nc():
    nc = bacc_mod.Bacc()
    x_in = nc.declare_dram_parameter("x", [P, NPOS], f32, isOutput=False)
    off_in = nc.declare_dram_parameter("offset", [18, NPOS], f32, isOutput=False)
    w_in = nc.declare_dram_parameter("weight", [P, 1152], f32, isOutput=False)
    out = nc.declare_dram_parameter("out", [P, NPOS], f32, isOutput=True)

    with tile.TileContext(nc) as tc:
        with tc.tile_pool(name="const", bufs=1) as cpool, \
             tc.tile_pool(name="persist", bufs=1) as ppool, \
             tc.tile_pool(name="dram", bufs=1, space="DRAM") as dpool:
            x_t = dpool.tile([NPOS, P], f16, name="x_t")
            w_rows = dpool.tile([36, NPOS], f16, name="w_rows")
            idx_rows = dpool.tile([18, NPOS], i16, name="idx_rows")
            # ---------- constants ----------
            ident16 = cpool.tile([P, P], f16)
            ident32 = cpool.tile([P, P], f32)
            ones_row = cpool.tile([1, P], f16)
            nc.vector.memset(ones_row[:], 1.0)
            onesP = cpool.tile([P, P], f32)
            nc.vector.memset(onesP[:], 1.0)
            ramp128 = cpool.tile([P, P], f32)
            nc.vector.tensor_tensor_scan(ramp128[:], onesP[:], onesP[:], -1.0,
                                         Alu.mult, Alu.add)
            # pcol[p] = p via DRAM bounce (partition-spread load)
            pcol_d = dpool.tile([1, P], f32, name="pcol_d")
            nc.sync.dma_start(pcol_d[:], ramp128[0:1, :])
            pcol = cpool.tile([P, 1], f32)
            src_p = AP(tensor=_h(pcol_d), offset=0, ap=[[1, P], [1, 1]])
            nc.sync.dma_start(pcol[:], src_p)
            nc.vector.tensor_scalar(ident32[:], ramp128[:], pcol[:], None,
                                    Alu.is_equal)
            nc.vector.tensor_copy(ident16[:], ident32[:])

            if STAGE == "K2":
                zk = ppool.tile([P, 3 * P], f32, name="zk")
                nc.vector.tensor_copy(zk[:, 0:P], ident32[:])
                nc.vector.tensor_copy(zk[:, P:2 * P], ramp128[:])
                nc.vector.tensor_copy(zk[:, 2 * P:2 * P + 1], pcol[:])
                nc.sync.dma_start(out[:, 0:3 * P], zk[:])
                zk2 = ppool.tile([P, NPOS - 3 * P], f32, name="zk2")
                nc.vector.memset(zk2[:], 0.0)
                nc.sync.dma_start(out[:, 3 * P:], zk2[:])
            # ---------- persistent tiles ----------
            idxw = ppool.tile([P, 18 * 576], i16)
            WkT = ppool.tile([P, K * P], f16)

            # ---------- phase A: x -> x_t, weights -> WkT ----------
            with tc.tile_pool(name="prepA", bufs=2) as pa, \
                 tc.tile_pool(name="prepAp", bufs=3, space="PSUM") as pap:
                x_sb = pa.tile([P, NPOS], f32, tag="xsb")
                nc.sync.dma_start(x_sb[:], x_in[:])
                x16 = pa.tile([P, NPOS], f16, tag="x16")
                nc.scalar.copy(x16[:], x_sb[:])
                for tq in range(NT // 4):
                    pt4 = pap.tile([P, 4 * P], f16, tag="pt4")
                    for j in range(4):
                        t = tq * 4 + j
                        nc.tensor.transpose(pt4[:, j * P:(j + 1) * P],
                                            x16[:, t * P:(t + 1) * P], ident16[:])
                    ev = pa.tile([P, 4 * P], f16, tag="ev")
                    nc.scalar.copy(ev[:], pt4[:])
                    dst = AP(tensor=_h(x_t), offset=tq * 512 * P,
                             ap=[[P, P], [128 * P, 4], [1, P]])
                    nc.sync.dma_start(dst, ev[:].rearrange("r (j c) -> r j c", j=4))

                w_sb = pa.tile([P, 1152], f32, tag="wsb")
                nc.sync.dma_start(w_sb[:], w_in[:])
                w16 = pa.tile([P, 1152], f16, tag="w16")
                nc.scalar.copy(w16[:], w_sb[:])
                for k in range(K):
                    wkc = pa.tile([P, P], f16, tag="wkc")
                    nc.vector.tensor_copy(wkc[:], w16[:, k:1152:9])
                    ptw = pap.tile([P, P], f16, tag="ptw")
                    nc.tensor.transpose(ptw[:], wkc[:], ident16[:])
                    nc.scalar.copy(WkT[:, k * P:(k + 1) * P], ptw[:])

            # ---------- phases B-D ----------
            if STAGE == "A":
                zo = ppool.tile([P, NPOS], f32)
                nc.vector.memset(zo[:], 0.0)
                nc.sync.dma_start(out[:], zo[:])
            if STAGE != "A":
                with tc.tile_pool(name="prepB", bufs=1) as pb, \
                     tc.tile_pool(name="prepBp", bufs=2, space="PSUM") as pbp:
                    off_sb = pb.tile([18, NPOS], f32, tag="offsb")
                    nc.sync.dma_start(off_sb[:], off_in[:])
                    offt = pb.tile([P, NT * 18], f32, tag="offt")
                    for tg in range(3):
                        pso = pbp.tile([P, 24 * 18], f32, tag="pso")
                        for j in range(24):
                            t = tg * 24 + j
                            nc.tensor.transpose(pso[:, j * 18:(j + 1) * 18],
                                                off_sb[0:18, t * P:(t + 1) * P],
                                                ident32[0:18, 0:18])
                        nc.vector.tensor_copy(offt[:, tg * 432:(tg + 1) * 432], pso[:])

                    # ---------- phase C: math ----------
                    NF = K * NT  # 648

                    def mt(tag, dt=f32):
                        return pb.tile([P, NF], dt, tag=tag, name=tag)

                    posf = pb.tile([P, NT], f32, tag="posf")
                    nc.vector.tensor_scalar(posf[:], ramp128[:, 0:NT], 128.0, None,
                                            Alu.mult)
                    nc.vector.tensor_scalar(posf[:], posf[:], pcol[:], None, Alu.add)

                    q0i = pb.tile([P, NT], i32, tag="q0i")
                    tmpq = pb.tile([P, NT], f32, tag="tmpq")
                    nc.vector.tensor_scalar(tmpq[:], posf[:], 1.0 / 96.0, None, Alu.mult)
                    nc.vector.tensor_copy(q0i[:], tmpq[:])
                    q0 = pb.tile([P, NT], f32, tag="q0")
                    nc.vector.tensor_copy(q0[:], q0i[:])
                    r0 = pb.tile([P, NT], f32, tag="r0")
                    nc.vector.scalar_tensor_tensor(r0[:], q0[:], -96.0, posf[:],
                                                   Alu.mult, Alu.add)
                    ltz = pb.tile([P, NT], f32, tag="ltz")
                    nc.vector.tensor_scalar(ltz[:], r0[:], 0.0, None, Alu.is_lt)
                    gez = pb.tile([P, NT], f32, tag="gez")
                    nc.vector.tensor_scalar(gez[:], r0[:], 96.0, None, Alu.is_ge)
                    Rr = pb.tile([P, NT], f32, tag="Rr")
                    nc.vector.tensor_tensor(Rr[:], q0[:], ltz[:], Alu.subtract)
                    nc.vector.tensor_tensor(Rr[:], Rr[:], gez[:], Alu.add)
                    Cc = pb.tile([P, NT], f32, tag="Cc")
                    nc.vector.scalar_tensor_tensor(Cc[:], ltz[:], 96.0, r0[:],
                                                   Alu.mult, Alu.add)
                    nc.vector.scalar_tensor_tensor(Cc[:], gez[:], -96.0, Cc[:],
                                                   Alu.mult, Alu.add)

                    BY = mt("BY")
                    BX = mt("BX")
                    for k in range(K):
                        ky, kx = k // 3, k % 3
                        nc.vector.tensor_scalar(BY[:, k * NT:(k + 1) * NT], Rr[:],
                                                float(ky - 1), None, Alu.add)
                        nc.vector.tensor_scalar(BX[:, k * NT:(k + 1) * NT], Cc[:],
                                                float(kx - 1), None, Alu.add)

                    offv = offt[:].rearrange("p (t pl) -> p pl t", pl=18)
                    py = mt("py")
                    px = mt("px")
                    nc.vector.tensor_tensor(
                        py[:].rearrange("p (k t) -> p k t", k=K),
                        offv[:, 0:18:2, :],
                        BY[:].rearrange("p (k t) -> p k t", k=K), Alu.add)
                    nc.vector.tensor_tensor(
                        px[:].rearrange("p (k t) -> p k t", k=K),
                        offv[:, 1:18:2, :],
                        BX[:].rearrange("p (k t) -> p k t", k=K), Alu.add)

                    def floor_frac(v, pfx):
                        vi = mt(pfx + "i", i32)
                        nc.vector.tensor_copy(vi[:], v[:])
                        vf = mt(pfx + "f")
                        nc.vector.tensor_copy(vf[:], vi[:])
                        fr = mt(pfx + "fr")
                        nc.vector.tensor_tensor(fr[:], v[:], vf[:], Alu.subtract)
                        ng = mt(pfx + "ng")
                        nc.vector.tensor_scalar(ng[:], fr[:], 0.0, None, Alu.is_lt)
                        nc.vector.tensor_tensor(vf[:], vf[:], ng[:], Alu.subtract)
                        nc.vector.tensor_tensor(fr[:], fr[:], ng[:], Alu.add)
                        return vf, fr

                    y0, fy = floor_frac(py, "y")
                    x0, fx = floor_frac(px, "x")

                    def range_mask(v, lo, hi, pfx):
                        g = mt(pfx + "g")
                        nc.vector.tensor_scalar(g[:], v[:], float(lo), None, Alu.is_ge)
                        l = mt(pfx + "l")
                        nc.vector.tensor_scalar(l[:], v[:], float(hi), None, Alu.is_le)
                        nc.vector.tensor_tensor(g[:], g[:], l[:], Alu.mult)
                        return g

                    vt = range_mask(y0, 0, 95, "vt")
                    vb = range_mask(y0, -1, 94, "vb")
                    inr = range_mask(x0, 0, 94, "inr")
                    omfy = mt("omfy")
                    nc.vector.tensor_scalar(omfy[:], fy[:], -1.0, 1.0, Alu.mult, Alu.add)
                    omfx = mt("omfx")
                    nc.vector.tensor_scalar(omfx[:], fx[:], -1.0, 1.0, Alu.mult, Alu.add)
                    wtop = mt("wtop")
                    nc.vector.tensor_tensor(wtop[:], omfy[:], vt[:], Alu.mult)
                    wbot = mt("wbot")
                    nc.vector.tensor_tensor(wbot[:], fy[:], vb[:], Alu.mult)
                    em1 = mt("em1")
                    nc.vector.tensor_scalar(em1[:], x0[:], -1.0, None, Alu.is_equal)
                    e95 = mt("e95")
                    nc.vector.tensor_scalar(e95[:], x0[:], 95.0, None, Alu.is_equal)
                    s0 = mt("s0")
                    s1 = mt("s1")
                    tmp = mt("tmp")
                    nc.vector.tensor_tensor(s0[:], inr[:], omfx[:], Alu.mult)
                    nc.vector.tensor_tensor(tmp[:], em1[:], fx[:], Alu.mult)
                    nc.vector.tensor_tensor(s0[:], s0[:], tmp[:], Alu.add)
                    nc.vector.tensor_tensor(s1[:], inr[:], fx[:], Alu.mult)
                    nc.vector.tensor_tensor(tmp[:], e95[:], omfx[:], Alu.mult)
                    nc.vector.tensor_tensor(s1[:], s1[:], tmp[:], Alu.add)

                    A0 = mt("A0", f16)
                    A1 = mt("A1", f16)
                    B0 = mt("B0", f16)
                    B1 = mt("B1", f16)
                    nc.vector.tensor_tensor(A0[:], wtop[:], s0[:], Alu.mult)
                    nc.vector.tensor_tensor(A1[:], wtop[:], s1[:], Alu.mult)
                    nc.vector.tensor_tensor(B0[:], wbot[:], s0[:], Alu.mult)
                    nc.vector.tensor_tensor(B1[:], wbot[:], s1[:], Alu.mult)

                    x0c = mt("x0c")
                    nc.vector.tensor_scalar(x0c[:], x0[:], 0.0, 94.0, Alu.max, Alu.min)
                    y0c = mt("y0c")
                    nc.vector.tensor_scalar(y0c[:], y0[:], 0.0, 95.0, Alu.max, Alu.min)
                    y1p = mt("y1p")
                    nc.vector.tensor_scalar(y1p[:], y0[:], -1.0, 94.0, Alu.max, Alu.min)
                    x0c96 = mt("x0c96")
                    nc.vector.tensor_scalar(x0c96[:], x0c[:], 96.0, None, Alu.add)
                    IDXT = mt("IDXT")
                    nc.vector.scalar_tensor_tensor(IDXT[:], y0c[:], 96.0, x0c[:],
                                                   Alu.mult, Alu.add)
                    IDXB = mt("IDXB")
                    nc.vector.scalar_tensor_tensor(IDXB[:], y1p[:], 96.0, x0c96[:],
                                                   Alu.mult, Alu.add)

                    # ---------- phase D ----------
                    for k in range(K):
                        psw = pbp.tile([NT, 4 * P], f16, tag="psw")
                        for s, tt_ in enumerate((A0, A1, B0, B1)):
                            nc.tensor.transpose(psw[:, s * P:(s + 1) * P],
                                                tt_[:, k * NT:(k + 1) * NT],
                                                ident16[:])
                        evw = pb.tile([NT, 4 * P], f16, tag="evw")
                        nc.scalar.copy(evw[:], psw[:])
                        dstw = AP(tensor=_h(w_rows), offset=(4 * k) * NPOS,
                                  ap=[[P, NT], [NPOS, 4], [1, P]])
                        nc.sync.dma_start(dstw,
                                          evw[:].rearrange("c (s e) -> c s e", s=4))

                        psi = pbp.tile([NT, 2 * P], f32, tag="psi")
                        nc.tensor.transpose(psi[:, 0:P],
                                            IDXT[:, k * NT:(k + 1) * NT], ident32[:])
                        nc.tensor.transpose(psi[:, P:2 * P],
                                            IDXB[:, k * NT:(k + 1) * NT], ident32[:])
                        evi = pb.tile([NT, 2 * P], i16, tag="evi")
                        nc.vector.tensor_copy(evi[:], psi[:])
                        dsti = AP(tensor=_h(idx_rows), offset=(2 * k) * NPOS,
                                  ap=[[P, NT], [NPOS, 2], [1, P]])
                        nc.sync.dma_start(dsti,
                                          evi[:].rearrange("c (s e) -> c s e", s=2))

            if STAGE == "D2":
                ird = ppool.tile([18, NPOS], i16, name="ird")
                nc.sync.dma_start(ird[:], idx_rows[:])
                irf = ppool.tile([18, NPOS], f32, name="irf")
                nc.vector.tensor_copy(irf[:], ird[:])
                nc.sync.dma_start(out[0:18, :], irf[:])
            # ---------- phase E ----------
            if STAGE == "B":
                zo = ppool.tile([P, NPOS], f32)
                nc.vector.memset(zo[:], 0.0)
                nc.sync.dma_start(out[:], zo[:])
            if STAGE in ("E", "E2", "F1a", "F1b", "F1c", "F1", "F"):
                nc.vector.memset(idxw[:], 0)
                for q in range(18):
                    src = AP(tensor=_h(idx_rows), offset=q * NPOS,
                             ap=[[1, 16], [16, 576]])
                    nc.sync.dma_start(idxw[0:16, q * 576:(q + 1) * 576], src)
                nc.sync.dma_start(idxw[16:32, :], idxw[0:16, :])
                nc.sync.dma_start(idxw[32:64, :], idxw[0:32, :])
                nc.sync.dma_start(idxw[64:128, :], idxw[0:64, :])

            # ---------- phase F: main loop ----------
            if STAGE == "E2":
                zo2 = ppool.tile([P, NPOS], f32)
                nc.vector.tensor_copy(zo2[:], idxw[:, 0:NPOS])
                nc.sync.dma_start(out[:], zo2[:])
            if STAGE == "E":
                zo = ppool.tile([P, NPOS], f32)
                nc.vector.memset(zo[:], 0.0)
                nc.sync.dma_start(out[:], zo[:])
            n_chunks = NCH if STAGE == "F" else 1
            xt_win = AP(tensor=_h(x_t), offset=0, ap=[[P, NROW], [1, 2 * P]])
            if STAGE in ("F1", "F"):
                n_chunks = NCH if STAGE == "F" else 1
                xt_win = AP(tensor=_h(x_t), offset=0, ap=[[P, NROW], [1, 2 * P]])
                with tc.tile_pool(name="g", bufs=int(os.environ.get("GB", "4"))) as gp, \
                     tc.tile_pool(name="aw", bufs=int(os.environ.get("AB", "3"))) as awp, \
                     tc.tile_pool(name="c4", bufs=int(os.environ.get("CB", "3"))) as c4p, \
                     tc.tile_pool(name="ops", bufs=2) as osp, \
                     tc.tile_pool(name="awps", bufs=2, space="PSUM") as awps, \
                     tc.tile_pool(name="outps", bufs=int(os.environ.get("OB", "1")), space="PSUM") as outps:
                    for c in range(n_chunks):
                        out_ps = outps.tile([P, CW], f32, tag="ops", name="out_ps")
                        for k in range(K):
                            ghs = []
                            for h in range(CW // GW):
                                gTh = gp.tile([P, 2, GW], f16, tag="gT", name="gTh")
                                gBh = gp.tile([P, 2, GW], f16, tag="gB", name="gBh")
                                i0t = (2 * k) * 576 + (c * CW + h * GW) // 16
                                i0b = (2 * k + 1) * 576 + (c * CW + h * GW) // 16
                                qn = ((2 * k + h) % 8) if os.environ.get("QN") == "spread" else 0
                                nc.gpsimd.dma_gather(
                                    gTh[:], xt_win,
                                    idxw[:, i0t:i0t + GW // 16],
                                    num_idxs=GW, num_idxs_reg=GW,
                                    elem_size=2 * P, elem_step=P, transpose=True,
                                    queue_num=qn)
                                nc.gpsimd.dma_gather(
                                    gBh[:], xt_win,
                                    idxw[:, i0b:i0b + GW // 16],
                                    num_idxs=GW, num_idxs_reg=GW,
                                    elem_size=2 * P, elem_step=P, transpose=True,
                                    queue_num=(qn + 4) % 8 if qn else 0)
                                ghs.append((gTh, gBh))
                            wst = awp.tile([1, 4 * CW], f16, tag="wst")
                            wsrc = AP(tensor=_h(w_rows),
                                      offset=(4 * k) * NPOS + c * CW,
                                      ap=[[NPOS, 4], [1, CW]])
                            nc.sync.dma_start(
                                wst[:].rearrange("p (s e) -> p s e", s=4),
                                wsrc.unsqueeze(0))
                            aw = awp.tile([P, 4, CW], f16, tag="aw")
                            if os.environ.get("AW_MODE", "pe") == "dma":
                                awsrc = AP(tensor=_h(w_rows),
                                           offset=(4 * k) * NPOS + c * CW,
                                           ap=[[0, P], [NPOS, 4], [1, CW]])
                                nc.scalar.dma_start(aw[:], awsrc)
                            else:
                                for si in range(4):
                                    import os as _os
                                    if _os.environ.get("OB", "1") == "2":
                                        for g3 in range(CW // SUB):
                                            psb = awps.tile([P, SUB], f32, tag="awps",
                                                            name="psb")
                                            base = si * CW + g3 * SUB
                                            nc.tensor.matmul(
                                                psb[:], ones_row[:],
                                                wst[0:1, base:base + SUB],
                                                start=True, stop=True)
                                            if (si + g3) % 2 == 0:
                                                nc.scalar.copy(
                                                    aw[:, si, g3 * SUB:(g3 + 1) * SUB], psb[:])
                                            else:
                                                nc.vector.tensor_copy(
                                                    aw[:, si, g3 * SUB:(g3 + 1) * SUB], psb[:])
                                    else:
                                        for h in range(CW // GW):
                                            psb = awps.tile([P, GW], f32, tag="awps",
                                                            name="psb")
                                            base = si * CW + h * GW
                                            nc.tensor.matmul(
                                                psb[:, 0:SUB], ones_row[:],
                                                wst[0:1, base:base + SUB],
                                                start=True, stop=True)
                                            nc.tensor.matmul(
                                                psb[:, SUB:GW], ones_row[:],
                                                wst[0:1, base + SUB:base + GW],
                                                start=True, stop=True)
                                            if (si + h) % 2 == 0:
                                                nc.scalar.copy(
                                                    aw[:, si, h * GW:(h + 1) * GW], psb[:])
                                            else:
                                                nc.vector.tensor_copy(
                                                    aw[:, si, h * GW:(h + 1) * GW], psb[:])
                            c4 = c4p.tile([P, 4, CW], f16, tag="c4")
                            for h, (gTh, gBh) in enumerate(ghs):
                                hs = slice(h * GW, (h + 1) * GW)
                                nc.vector.tensor_tensor(
                                    c4[:, 0:2, hs], gTh[:],
                                    aw[:, 0:2, hs], Alu.mult)
                                nc.vector.tensor_tensor(
                                    c4[:, 2:4, hs], gBh[:],
                                    aw[:, 2:4, hs], Alu.mult)
                            for j in range(CW // SUB):
                                for si in range(4):
                                    nc.tensor.matmul(
                                        out_ps[:, j * SUB:(j + 1) * SUB],
                                        WkT[:, k * P:(k + 1) * P],
                                        c4[:, si, j * SUB:(j + 1) * SUB],
                                        start=(k == 0 and si == 0),
                                        stop=(k == K - 1 and si == 3),
                                        skip_group_check=True)
                        osb = osp.tile([P, CW], f32, tag="osb")
                        nc.vector.tensor_copy(osb[:], out_ps[:])
                        nc.sync.dma_start(out[:, c * CW:(c + 1) * CW], osb[:])
    nc.compile()
    return nc


_NC = None


def kernel(x, offset, weight):
    global _NC
    if _NC is None:
        _NC = build_nc()
    from concourse.bass_utils import run_bass_kernel_spmd
    B = x.shape[0]
    w2 = np.ascontiguousarray(weight.reshape(P, 1152)).astype(np.float32)
    in_maps = []
    for b in range(B):
        in_maps.append({
            "x": np.ascontiguousarray(np.asarray(x)[b].reshape(P, NPOS), dtype=np.float32),
            "offset": np.ascontiguousarray(np.asarray(offset)[b].reshape(18, NPOS), dtype=np.float32),
            "weight": w2,
        })
    res = run_bass_kernel_spmd(_NC, in_maps, list(range(B)))
    outs = [res.results[b]["out"].reshape(P, H, W) for b in range(B)]
    return np.stack(outs).astype(np.float32)



# revision 64
# speedup vs baseline: 1.5017x; 1.5017x over previous
"""DeformConv2d forward on 8 Trainium2 NeuronCores (Bass/Tile).

x[8,128,96,96] f32, offset[8,18,96,96] f32, weight[128,128,3,3] f32
-> out[8,128,96,96] f32. Deformable 3x3 conv, pad 1, stride 1, bilinear
sampling with zero padding. Data-parallel over batch: one element per core.

Per-core pipeline (v2):
  A. x -> f16 into a zero-padded 98x98 image (SBUF, channel-major),
     PE-transposed to pixel-major x_tp[9728,128] f16 in DRAM. Zero padding
     makes per-corner validity automatic; only a single range mask remains.
  B. offsets PE-transposed to a position-packed layout.
  C. DVE index/weight math in [128, 9*72] packed layout: corner weights
     A0,A1,B0,B1 (f16) and padded row indices jT=(y0c+1)*98+(x0c+1),
     jB=jT+98 (clamped in-range; out-of-range samples get zero weight).
  D. PE-transpose weights/indices to row-major DRAM; indices stored
     16-partition-wrap-major so phase E loads are contiguous.
  E. idx_rows -> wrapped SBUF layout for dma_gather.
  F. Main loop per (chunk of 1536 positions, tap): two 1536-index gathers
     (top/bottom pixel pairs, channels on partitions); PE broadcasts slot
     weights via ones-matmul into f32 PSUM; Act(+some DVE) evacuates to f16
     SBUF; DVE multiplies gathered pairs by slot weights (f16 2x mode); PE
     GEMM accumulates over (ci, tap, slot) in PSUM.
"""
import sys
if '/opt/trn_rl_repo' not in sys.path:
    sys.path.insert(0, '/opt/trn_rl_repo')

import os

import numpy as np

import concourse.bacc as bacc_mod
import concourse.mybir as mybir
import concourse.tile as tile
from concourse.ap import AP

f32 = mybir.dt.float32
f16 = mybir.dt.float16
i16 = mybir.dt.int16
i32 = mybir.dt.int32
Alu = mybir.AluOpType

P = 128
H = W = 96
NPOS = H * W              # 9216
NT = NPOS // P            # 72 position tiles
K = 9
NF = K * NT               # 648
PW = 98                   # padded image row width
NTP = 76                  # padded-image transpose tiles (76*128 = 9728)
NPADR = NTP * P           # 9728 rows in x_tp
CW = 1536                 # main-loop position chunk
NCH = NPOS // CW          # 6 chunks
HB = 512                  # PSUM sub-block (one f32 bank)


def _h(ap_or_handle):
    return ap_or_handle.tensor if hasattr(ap_or_handle, 'tensor') else ap_or_handle


def build_nc():
    nc = bacc_mod.Bacc(dynamic_dma_scratch_size=65536)
    x_in = nc.declare_dram_parameter("x", [P, NPOS], f32, isOutput=False)
    off_in = nc.declare_dram_parameter("offset", [18, NPOS], f32, isOutput=False)
    w_in = nc.declare_dram_parameter("weight", [P, 1152], f32, isOutput=False)
    out = nc.declare_dram_parameter("out", [P, NPOS], f32, isOutput=True)

    with tile.TileContext(nc) as tc:
        with tc.tile_pool(name="const", bufs=1) as cpool, \
             tc.tile_pool(name="persist", bufs=1) as ppool, \
             tc.tile_pool(name="dram", bufs=1, space="DRAM") as dpool:
            # x_cp[j] = [x_pad[row j], x_pad[row j+98]] (vertical pair):
            # one 1KB gather element covers the whole 2x2 bilinear quad.
            x_cp = dpool.tile([NPADR, 2 * P], f16, name="x_cp")
            w_rows = dpool.tile([36, NPOS], f16, name="w_rows")
            idx_rows = dpool.tile([K, NPOS], i16, name="idx_rows")
            # ---------- constants ----------
            ident16 = cpool.tile([P, P], f16)
            ident32 = cpool.tile([P, P], f32)
            ones1 = cpool.tile([1, P], f16)
            nc.vector.memset(ones1[:], 1.0)
            ones3 = cpool.tile([65, P], f16)
            nc.vector.memset(ones3[:], 1.0)
            onesP = cpool.tile([P, P], f32)
            nc.vector.memset(onesP[:], 1.0)
            ramp128 = cpool.tile([P, P], f32)
            nc.vector.tensor_tensor_scan(ramp128[:], onesP[:], onesP[:], -1.0,
                                         Alu.mult, Alu.add)
            pcol_d = dpool.tile([1, P], f32, name="pcol_d")
            nc.sync.dma_start(pcol_d[:], ramp128[0:1, :])
            pcol = cpool.tile([P, 1], f32)
            src_p = AP(tensor=_h(pcol_d), offset=0, ap=[[1, P], [1, 1]])
            nc.sync.dma_start(pcol[:], src_p)
            nc.vector.tensor_scalar(ident32[:], ramp128[:], pcol[:], None,
                                    Alu.is_equal)
            nc.vector.tensor_copy(ident16[:], ident32[:])
            # wrap-permutation matrix: Pprm[p, f] = 1 iff p = (f%8)*16 + f//8.
            # Used for the index transposes so idx stores land wrap-major.
            prow = cpool.tile([1, P], f32)
            prowi = cpool.tile([1, P], i32)
            nc.vector.tensor_scalar(prow[:], ramp128[0:1, :], 1.0 / 16.0,
                                    None, Alu.mult)
            nc.vector.tensor_copy(prowi[:], prow[:])
            nc.vector.tensor_copy(prow[:], prowi[:])          # q16 = c//16
            ppd = dpool.tile([1, P], f32, name="ppd")
            prow2 = cpool.tile([1, P], f32)
            # val = 8*(c - 16*q16) + q16 = 8*c - 128*q16 + q16
            nc.vector.tensor_scalar(prow2[:], prow[:], -127.0, None, Alu.mult)
            nc.vector.scalar_tensor_tensor(prow2[:], ramp128[0:1, :], 8.0,
                                           prow2[:], Alu.mult, Alu.add)
            nc.sync.dma_start(ppd[:], prow2[:])
            pcolI = cpool.tile([P, 1], f32)
            src_pi = AP(tensor=_h(ppd), offset=0, ap=[[1, P], [1, 1]])
            nc.sync.dma_start(pcolI[:], src_pi)
            Pprm32 = cpool.tile([P, P], f32)
            nc.vector.tensor_scalar(Pprm32[:], ramp128[:], pcolI[:], None,
                                    Alu.is_equal)

            # ---------- persistent tiles ----------
            idxw = ppool.tile([P, K * 576], i16)
            WkT = ppool.tile([P, K * P], f16)

            with tc.tile_pool(name="prepA", bufs=2) as pa:
                x16p = pa.tile([P, NPADR], f16, tag="x16p")
                w16 = pa.tile([P, 1152], f16, tag="w16")
                offt = pa.tile([P, NT * 18], f32, tag="offt")
                # ---- loads first (in-order DMA queue) ----
                with tc.tile_pool(name="ld", bufs=1) as pld, \
                     tc.tile_pool(name="psoP", bufs=2, space="PSUM") as psoP:
                    x_sb = pld.tile([P, NPOS], f32, tag="xsb")
                    nc.sync.dma_start(x_sb[:], x_in[:])
                    off_sb = pld.tile([18, NPOS], f32, tag="offsb")
                    nc.sync.dma_start(off_sb[:], off_in[:])
                    w_sb = pld.tile([P, 1152], f32, tag="wsb")
                    nc.sync.dma_start(w_sb[:], w_in[:])

                    nc.gpsimd.memset(x16p[:], 0.0)
                    # interior: x16p[:, (y+1)*98 + (x+1)] = f16(x[:, y*96+x])
                    dst = x16p[:, PW:PW + H * PW].rearrange(
                        "p (r w) -> p r w", w=PW)[:, :, 1:1 + W]
                    srcx = x_sb[:].rearrange("p (r w) -> p r w", w=W)
                    nc.scalar.copy(dst[:, 0:H // 2, :], srcx[:, 0:H // 2, :])
                    nc.vector.tensor_copy(dst[:, H // 2:, :],
                                          srcx[:, H // 2:, :])

                    for tg in range(3):
                        pso = psoP.tile([P, 24 * 18], f32, tag="pso")
                        for j in range(24):
                            t = tg * 24 + j
                            nc.tensor.transpose(pso[:, j * 18:(j + 1) * 18],
                                                off_sb[0:18, t * P:(t + 1) * P],
                                                ident32[0:18, 0:18])
                        nc.scalar.copy(offt[:, tg * 432:(tg + 1) * 432], pso[:])

                    nc.scalar.copy(w16[:], w_sb[:])

                # ---- phase A: transpose padded image to pixel-major ----
                # Each pixel row j is stored twice: as the top half of pair
                # j and the bottom half of pair j-98.
                with tc.tile_pool(name="ptP", bufs=2, space="PSUM") as ptP:
                    groups = [8] * 9 + [4]
                    t0 = 0
                    for gi, gw in enumerate(groups):
                        pt8 = ptP.tile([P, 8 * P], f16, tag="pt8")
                        for j in range(gw):
                            t = t0 + j
                            nc.tensor.transpose(pt8[:, j * P:(j + 1) * P],
                                                x16p[:, t * P:(t + 1) * P],
                                                ident16[:])
                        ev = pa.tile([P, 8 * P], f16, tag="ev")
                        if gi % 4 == 3:
                            nc.vector.tensor_copy(ev[:, 0:gw * P],
                                                  pt8[:, 0:gw * P])
                        else:
                            nc.scalar.copy(ev[:, 0:gw * P], pt8[:, 0:gw * P])
                        dst1 = AP(tensor=_h(x_cp), offset=t0 * P * 2 * P,
                                  ap=[[2 * P, P], [128 * 2 * P, gw], [1, P]])
                        nc.sync.dma_start(
                            dst1,
                            ev[:, 0:gw * P].rearrange("r (j c) -> r j c",
                                                      j=gw))
                        if gi == 0:
                            dst2a = AP(tensor=_h(x_cp), offset=P,
                                       ap=[[2 * P, 30], [1, P]])
                            nc.sync.dma_start(dst2a, ev[98:128, 0:P])
                            dst2b = AP(tensor=_h(x_cp),
                                       offset=30 * 2 * P + P,
                                       ap=[[2 * P, P], [128 * 2 * P, gw - 1],
                                           [1, P]])
                            nc.sync.dma_start(
                                dst2b,
                                ev[:, P:gw * P].rearrange(
                                    "r (j c) -> r j c", j=gw - 1))
                        else:
                            dst2 = AP(tensor=_h(x_cp),
                                      offset=(t0 * P - 98) * 2 * P + P,
                                      ap=[[2 * P, P], [128 * 2 * P, gw],
                                          [1, P]])
                            nc.sync.dma_start(
                                dst2,
                                ev[:, 0:gw * P].rearrange("r (j c) -> r j c",
                                                          j=gw))
                        t0 += gw
                    # zero the never-paired tail bottom halves (pairs
                    # 9630..9727) so the gather window view is fully defined
                    ztail = AP(tensor=_h(x_cp),
                               offset=(NPADR - 98) * 2 * P + P,
                               ap=[[2 * P, 98], [1, P]])
                    nc.sync.dma_start(ztail, x16p[0:98, 9728 - P:9728])

                    # ---- conv weights -> WkT ----
                    for k in range(K):
                        wkc = pa.tile([P, P], f16, tag="wkc")
                        nc.scalar.copy(wkc[:], w16[:, k:1152:9])
                        ptw = ptP.tile([P, P], f16, tag="ptw")
                        nc.tensor.transpose(ptw[:], wkc[:], ident16[:])
                        nc.scalar.copy(WkT[:, k * P:(k + 1) * P], ptw[:])

                # ---- phase C: position/weight/index math (DVE) ----
                with tc.tile_pool(name="pc", bufs=1) as pc:
                    def st(tag, dt=f32):
                        return pc.tile([P, NT], dt, tag=tag, name=tag)

                    def mt(tag, dt=f32):
                        return pc.tile([P, NF], dt, tag=tag, name=tag)

                    posf = st("posf")
                    nc.vector.tensor_scalar(posf[:], ramp128[:, 0:NT], 128.0,
                                            None, Alu.mult)
                    nc.vector.tensor_scalar(posf[:], posf[:], pcol[:], None,
                                            Alu.add)
                    q0i = st("q0i", i32)
                    tmpq = st("tmpq")
                    nc.vector.tensor_scalar(tmpq[:], posf[:], 1.0 / 96.0, None,
                                            Alu.mult)
                    nc.vector.tensor_copy(q0i[:], tmpq[:])
                    q0 = st("q0")
                    nc.vector.tensor_copy(q0[:], q0i[:])
                    r0 = st("r0")
                    nc.vector.scalar_tensor_tensor(r0[:], q0[:], -96.0, posf[:],
                                                   Alu.mult, Alu.add)
                    ltz = st("ltz")
                    nc.vector.tensor_scalar(ltz[:], r0[:], 0.0, None, Alu.is_lt)
                    gez = st("gez")
                    nc.vector.tensor_scalar(gez[:], r0[:], 96.0, None, Alu.is_ge)
                    Rr = st("Rr")
                    nc.vector.tensor_tensor(Rr[:], q0[:], ltz[:], Alu.subtract)
                    nc.vector.tensor_tensor(Rr[:], Rr[:], gez[:], Alu.add)
                    Cc = st("Cc")
                    nc.vector.scalar_tensor_tensor(Cc[:], ltz[:], 96.0, r0[:],
                                                   Alu.mult, Alu.add)
                    nc.vector.scalar_tensor_tensor(Cc[:], gez[:], -96.0, Cc[:],
                                                   Alu.mult, Alu.add)

                    T1 = mt("T1")
                    T2 = mt("T2")
                    T3 = mt("T3")
                    T4 = mt("T4")
                    T5 = mt("T5")
                    T6 = mt("T6")
                    T7 = mt("T7")
                    T8 = mt("T8")
                    VI = mt("VI", i32)
                    A0 = mt("A0", f16)
                    A1 = mt("A1", f16)
                    B0 = mt("B0", f16)
                    B1 = mt("B1", f16)

                    for k in range(K):
                        ky, kx = k // 3, k % 3
                        nc.vector.tensor_scalar(T1[:, k * NT:(k + 1) * NT],
                                                Rr[:], float(ky - 1), None,
                                                Alu.add)
                        nc.vector.tensor_scalar(T2[:, k * NT:(k + 1) * NT],
                                                Cc[:], float(kx - 1), None,
                                                Alu.add)
                    offv = offt[:].rearrange("p (t pl) -> p pl t", pl=18)
                    # py (T1), px (T2)
                    nc.vector.tensor_tensor(
                        T1[:].rearrange("p (k t) -> p k t", k=K),
                        offv[:, 0:18:2, :],
                        T1[:].rearrange("p (k t) -> p k t", k=K), Alu.add)
                    nc.vector.tensor_tensor(
                        T2[:].rearrange("p (k t) -> p k t", k=K),
                        offv[:, 1:18:2, :],
                        T2[:].rearrange("p (k t) -> p k t", k=K), Alu.add)

                    def floor_frac(v, vf, fr, ng):
                        nc.vector.tensor_copy(VI[:], v[:])
                        nc.vector.tensor_copy(vf[:], VI[:])
                        nc.vector.tensor_tensor(fr[:], v[:], vf[:], Alu.subtract)
                        nc.vector.tensor_scalar(ng[:], fr[:], 0.0, None,
                                                Alu.is_lt)
                        nc.vector.tensor_tensor(vf[:], vf[:], ng[:],
                                                Alu.subtract)
                        nc.vector.tensor_tensor(fr[:], fr[:], ng[:], Alu.add)

                    floor_frac(T1, T3, T4, T7)   # y0=T3, fy=T4
                    floor_frac(T2, T5, T6, T7)   # x0=T5, fx=T6

                    # mask mm = (y0 in [-1,95]) & (x0 in [-1,95]) -> T8
                    nc.vector.tensor_scalar(T8[:], T3[:], -1.0, None, Alu.is_ge)
                    nc.vector.tensor_scalar(T1[:], T3[:], 95.0, None, Alu.is_le)
                    nc.vector.tensor_tensor(T8[:], T8[:], T1[:], Alu.mult)
                    nc.vector.tensor_scalar(T1[:], T5[:], -1.0, None, Alu.is_ge)
                    nc.vector.tensor_scalar(T2[:], T5[:], 95.0, None, Alu.is_le)
                    nc.vector.tensor_tensor(T1[:], T1[:], T2[:], Alu.mult)
                    nc.vector.tensor_tensor(T8[:], T8[:], T1[:], Alu.mult)

                    # wbot (T2) = fy*mm ; wtop (T1) = mm - wbot ; omfx (T7)
                    nc.vector.tensor_tensor(T2[:], T4[:], T8[:], Alu.mult)
                    nc.vector.tensor_tensor(T1[:], T8[:], T2[:], Alu.subtract)
                    nc.vector.tensor_scalar(T7[:], T6[:], -1.0, 1.0, Alu.mult,
                                            Alu.add)
                    nc.vector.tensor_tensor(A0[:], T1[:], T7[:], Alu.mult)
                    nc.vector.tensor_tensor(A1[:], T1[:], T6[:], Alu.mult)
                    nc.vector.tensor_tensor(B0[:], T2[:], T7[:], Alu.mult)
                    nc.vector.tensor_tensor(B1[:], T2[:], T6[:], Alu.mult)

                    # jT = (clip(y0)+1)*98 + clip(x0)+1 ; jB = jT + 98
                    nc.vector.tensor_scalar(T3[:], T3[:], -1.0, 95.0, Alu.max,
                                            Alu.min)
                    nc.vector.tensor_scalar(T5[:], T5[:], -1.0, 95.0, Alu.max,
                                            Alu.min)
                    nc.vector.tensor_scalar(T5[:], T5[:], 99.0, None, Alu.add)
                    nc.vector.scalar_tensor_tensor(T4[:], T3[:], 98.0, T5[:],
                                                   Alu.mult, Alu.add)   # JT

                    # ---- phase D: to row-major DRAM ----
                    # weight row order matches gather slot order:
                    # (y0x0, y1x0, y0x1, y1x1) -> (A0, B0, A1, B1)
                    with tc.tile_pool(name="prepDp", bufs=2,
                                      space="PSUM") as pdp:
                        for k in range(K):
                            psw = pdp.tile([NT, 4 * P], f16, tag="psw")
                            for s, tt_ in enumerate((A0, B0, A1, B1)):
                                nc.tensor.transpose(psw[:, s * P:(s + 1) * P],
                                                    tt_[:, k * NT:(k + 1) * NT],
                                                    ident16[:])
                            evw = pa.tile([NT, 4 * P], f16, tag="evw")
                            nc.scalar.copy(evw[:], psw[:])
                            dstw = AP(tensor=_h(w_rows), offset=(4 * k) * NPOS,
                                      ap=[[P, NT], [NPOS, 4], [1, P]])
                            nc.sync.dma_start(
                                dstw, evw[:].rearrange("c (s e) -> c s e", s=4))

                            psi = pdp.tile([NT, P], f32, tag="psi")
                            nc.tensor.transpose(psi[:],
                                                T4[:, k * NT:(k + 1) * NT],
                                                ident32[:])
                            evi0 = pa.tile([NT, P], i16, tag="evi0")
                            nc.vector.tensor_copy(evi0[:], psi[:])
                            # pre-permute on DVE so the wrap-major store has
                            # contiguous runs: evi[c, el*8+eh] = evi0[c, e]
                            # with e = eh*16+el
                            evi = pa.tile([NT, P], i16, tag="evi")
                            nc.vector.tensor_copy(
                                evi[:].rearrange("c (el eh) -> c el eh",
                                                 el=16, eh=8),
                                evi0[:].rearrange("c (eh el) -> c el eh",
                                                  eh=8, el=16))
                            # idx i=t*128+eh*16+el sits at evi free f=el*8+eh;
                            # store to w = el*576 + 8t + eh (wrap-major)
                            dsti = AP(tensor=_h(idx_rows),
                                      offset=k * NPOS,
                                      ap=[[8, NT], [576, 16], [1, 8]])
                            srci = evi[:].rearrange(
                                "c (el eh) -> c el eh", el=16, eh=8)
                            nc.sync.dma_start(dsti, srci)

                            # phase E (per k): wrapped load; k=0 replicated
                            # eagerly so the first gather starts early, the
                            # rest in three wide DMAs after the last tap.
                            srcq = AP(tensor=_h(idx_rows), offset=k * NPOS,
                                      ap=[[576, 16], [1, 576]])
                            eng = nc.gpsimd if k == 0 else nc.sync
                            eng.dma_start(
                                idxw[0:16, k * 576:(k + 1) * 576], srcq)
                            if k == 0:
                                ks = slice(0, 576)
                                nc.sync.dma_start(idxw[16:32, ks],
                                                  idxw[0:16, ks])
                                nc.sync.dma_start(idxw[32:64, ks],
                                                  idxw[0:32, ks])
                                nc.sync.dma_start(idxw[64:128, ks],
                                                  idxw[0:64, ks])
                        ks = slice(576, K * 576)
                        nc.sync.dma_start(idxw[16:32, ks], idxw[0:16, ks])
                        nc.sync.dma_start(idxw[32:64, ks], idxw[0:32, ks])
                        nc.sync.dma_start(idxw[64:128, ks], idxw[0:64, ks])

            # ---------- phase F: main loop ----------
            xt_win = AP(tensor=_h(x_cp), offset=0,
                        ap=[[2 * P, NPADR - 1], [1, 4 * P]])
            evac_ctr = 0
            with tc.tile_pool(name="g", bufs=int(os.environ.get("GB", "3"))) as gp, \
                 tc.tile_pool(name="wstp", bufs=2) as wstp, \
                 tc.tile_pool(name="wstPp", bufs=3) as wstPp, \
                 tc.tile_pool(name="aw16p", bufs=int(os.environ.get("AB", "2"))) as aw16p, \
                 tc.tile_pool(name="c4p", bufs=int(os.environ.get("CB", "2"))) as c4p, \
                 tc.tile_pool(name="osp", bufs=2) as osp, \
                 tc.tile_pool(name="awps", bufs=2, space="PSUM") as awps, \
                 tc.tile_pool(name="outps", bufs=1, space="PSUM") as outps:
                def iter_seq():
                    for c in range(NCH):
                        for k in range(K):
                            yield c, k

                GW = 768  # per-gather index count (hw limit <= 896)

                def issue_gather(c, k):
                    g4 = gp.tile([P, CW // GW, 4, GW], f16, tag="g4",
                                 name="g4")
                    for hh in range(CW // GW):
                        i0 = k * 576 + (c * CW + hh * GW) // 16
                        nc.gpsimd.dma_gather(
                            g4[:, hh], xt_win,
                            idxw[:, i0:i0 + GW // 16],
                            num_idxs=GW, num_idxs_reg=GW,
                            elem_size=4 * P, elem_step=2 * P, transpose=True)
                    return g4

                def issue_wst3(c, g):
                    # one DMA loads 3 taps' weight rows (12 rows) into a
                    # 3-partition tile; prefetched a group ahead so the small
                    # transfer isn't starved behind gathers on the DMA track
                    w3 = wstp.tile([65, 4, CW], f16, tag="wst3")
                    wsrc = AP(tensor=_h(w_rows),
                              offset=(4 * 3 * g) * NPOS + c * CW,
                              ap=[[4 * NPOS, 3], [NPOS, 4], [1, CW]])
                    nc.sync.dma_start(w3[0:65:32, :, :], wsrc)
                    return w3

                iters = list(iter_seq())
                wgroups = [(c, g) for c in range(NCH) for g in range(3)]
                pending_w = [issue_wst3(*wgroups[0])]
                wg_idx = 0

                uo_list = os.environ.get("UO", "ADAAPA").split("/")

                def p_unit_of(it):
                    order = uo_list[it % len(uo_list)]
                    for ui, u in enumerate(order):
                        if u == 'P':
                            return ui // 3, ui % 3
                    return None

                def issue_wstP(it):
                    pu = p_unit_of(it)
                    if pu is None:
                        return None
                    c, k = iters[it]
                    half, h = pu
                    wstP = wstPp.tile([1, 2, HB], f16, tag="wstP")
                    wpsrc = AP(tensor=_h(w_rows),
                               offset=(4 * k + 2 * half) * NPOS
                               + c * CW + h * HB,
                               ap=[[NPOS, 2], [1, HB]])
                    nc.sync.dma_start(wstP[:], wpsrc.unsqueeze(0))
                    return wstP

                pending_p = [issue_wstP(0), issue_wstP(1)]
                g4_next = issue_gather(*iters[0])
                for idx_it, (c, k) in enumerate(iters):
                    if k == 0:
                        out_ps = outps.tile([P, CW], f32, tag="ops",
                                            name="out_ps")
                    g4 = g4_next
                    if idx_it + 1 < len(iters):
                        g4_next = issue_gather(*iters[idx_it + 1])
                    if k % 3 == 0:
                        wst3 = pending_w.pop(0)
                        wg_idx += 1
                        if wg_idx < len(wgroups):
                            pending_w.append(issue_wst3(*wgroups[wg_idx]))
                    kk = k % 3
                    wstP_cur = pending_p.pop(0)
                    if idx_it + 2 < len(iters):
                        pending_p.append(issue_wstP(idx_it + 2))
                    if True:

                        # per half (top slots 0:2 / bottom 2:4): 3 aw units
                        # per half as separate contiguous [P,2,HB] tiles; the
                        # 'P' unit is a Pool partition_broadcast from a small
                        # partition-0 staging tile (HW requires contiguous
                        # partition-0 source).
                        unit_order = uo_list[idx_it % len(uo_list)]
                        awU = [[None] * (CW // HB) for _ in range(2)]
                        for half in (0, 1):
                            for h in range(CW // HB):
                                aw1 = aw16p.tile([P, 2, HB], f16,
                                                 tag=f"aw{half}{h}")
                                awU[half][h] = aw1
                                u = unit_order[evac_ctr % 6]
                                evac_ctr += 1
                                if u == 'P':
                                    nc.gpsimd.partition_broadcast(
                                        aw1[:], wstP_cur[:])
                                    continue
                                awp_t = awps.tile([P, 2, HB], f32, tag="awp")
                                for s2 in range(2):
                                    nc.tensor.matmul(
                                        awp_t[:, s2, :],
                                        ones3[32 * kk:32 * kk + 1, :],
                                        wst3[32 * kk:32 * kk + 1,
                                             2 * half + s2,
                                             h * HB:(h + 1) * HB],
                                        start=True, stop=True)
                                if u == 'D':
                                    nc.vector.tensor_copy(aw1[:], awp_t[:])
                                else:
                                    nc.scalar.copy(aw1[:], awp_t[:])

                        for half in (0, 1):
                            c4 = c4p.tile([P, 2, CW], f16, tag=f"c4{half}")
                            # weight segments: intersections of the 512-wide
                            # aw tiles with the 768-wide gather sub-blocks
                            for h in range(CW // HB):
                                p0 = h * HB
                                while p0 < (h + 1) * HB:
                                    hh = p0 // GW
                                    p1 = min((h + 1) * HB, (hh + 1) * GW)
                                    nc.vector.tensor_tensor(
                                        c4[:, :, p0:p1],
                                        g4[:, hh, 2 * half:2 * half + 2,
                                           p0 - hh * GW:p1 - hh * GW],
                                        awU[half][h][:, :,
                                                     p0 - h * HB:p1 - h * HB],
                                        Alu.mult)
                                    p0 = p1
                            for s2 in range(2):
                                for j in range(CW // HB):
                                    nc.tensor.matmul(
                                        out_ps[:, j * HB:(j + 1) * HB],
                                        WkT[:, k * P:(k + 1) * P],
                                        c4[:, s2, j * HB:(j + 1) * HB],
                                        start=(k == 0 and half == 0
                                               and s2 == 0),
                                        stop=(k == K - 1 and half == 1
                                              and s2 == 1),
                                        skip_group_check=True)
                    if k == K - 1:
                        osb = osp.tile([P, CW], f32, tag="osb")
                        nc.scalar.copy(osb[:], out_ps[:])
                        nc.sync.dma_start(out[:, c * CW:(c + 1) * CW], osb[:])
    nc.compile()
    return nc


_NC = None


def kernel(x, offset, weight):
    global _NC
    if _NC is None:
        _NC = build_nc()
    from concourse.bass_utils import run_bass_kernel_spmd
    B = x.shape[0]
    w2 = np.ascontiguousarray(np.asarray(weight).reshape(P, 1152)).astype(np.float32)
    in_maps = []
    for b in range(B):
        in_maps.append({
            "x": np.ascontiguousarray(np.asarray(x)[b].reshape(P, NPOS), dtype=np.float32),
            "offset": np.ascontiguousarray(np.asarray(offset)[b].reshape(18, NPOS), dtype=np.float32),
            "weight": w2,
        })
    res = run_bass_kernel_spmd(_NC, in_maps, list(range(B)))
    outs = [res.results[b]["out"].reshape(P, H, W) for b in range(B)]
    return np.stack(outs).astype(np.float32)


# revision 65
# speedup vs baseline: 1.5069x; 1.0035x over previous
"""DeformConv2d forward on 8 Trainium2 NeuronCores (Bass/Tile).

x[8,128,96,96] f32, offset[8,18,96,96] f32, weight[128,128,3,3] f32
-> out[8,128,96,96] f32. Deformable 3x3 conv, pad 1, stride 1, bilinear
sampling with zero padding. Data-parallel over batch: one element per core.

Per-core pipeline (v2):
  A. x -> f16 into a zero-padded 98x98 image (SBUF, channel-major),
     PE-transposed to pixel-major x_tp[9728,128] f16 in DRAM. Zero padding
     makes per-corner validity automatic; only a single range mask remains.
  B. offsets PE-transposed to a position-packed layout.
  C. DVE index/weight math in [128, 9*72] packed layout: corner weights
     A0,A1,B0,B1 (f16) and padded row indices jT=(y0c+1)*98+(x0c+1),
     jB=jT+98 (clamped in-range; out-of-range samples get zero weight).
  D. PE-transpose weights/indices to row-major DRAM; indices stored
     16-partition-wrap-major so phase E loads are contiguous.
  E. idx_rows -> wrapped SBUF layout for dma_gather.
  F. Main loop per (chunk of 1536 positions, tap): two 1536-index gathers
     (top/bottom pixel pairs, channels on partitions); PE broadcasts slot
     weights via ones-matmul into f32 PSUM; Act(+some DVE) evacuates to f16
     SBUF; DVE multiplies gathered pairs by slot weights (f16 2x mode); PE
     GEMM accumulates over (ci, tap, slot) in PSUM.
"""
import sys
if '/opt/trn_rl_repo' not in sys.path:
    sys.path.insert(0, '/opt/trn_rl_repo')

import os

import numpy as np

import concourse.bacc as bacc_mod
import concourse.mybir as mybir
import concourse.tile as tile
from concourse.ap import AP

f32 = mybir.dt.float32
f16 = mybir.dt.float16
i16 = mybir.dt.int16
i32 = mybir.dt.int32
Alu = mybir.AluOpType

P = 128
H = W = 96
NPOS = H * W              # 9216
NT = NPOS // P            # 72 position tiles
K = 9
NF = K * NT               # 648
PW = 98                   # padded image row width
NTP = 76                  # padded-image transpose tiles (76*128 = 9728)
NPADR = NTP * P           # 9728 rows in x_tp
CW = 1536                 # main-loop position chunk
NCH = NPOS // CW          # 6 chunks
HB = 512                  # PSUM sub-block (one f32 bank)


def _h(ap_or_handle):
    return ap_or_handle.tensor if hasattr(ap_or_handle, 'tensor') else ap_or_handle


def build_nc():
    nc = bacc_mod.Bacc(dynamic_dma_scratch_size=65536)
    x_in = nc.declare_dram_parameter("x", [P, NPOS], f32, isOutput=False)
    off_in = nc.declare_dram_parameter("offset", [18, NPOS], f32, isOutput=False)
    w_in = nc.declare_dram_parameter("weight", [P, 1152], f32, isOutput=False)
    out = nc.declare_dram_parameter("out", [P, NPOS], f32, isOutput=True)

    with tile.TileContext(nc) as tc:
        with tc.tile_pool(name="const", bufs=1) as cpool, \
             tc.tile_pool(name="persist", bufs=1) as ppool, \
             tc.tile_pool(name="dram", bufs=1, space="DRAM") as dpool:
            # x_cp[j] = [x_pad[row j], x_pad[row j+98]] (vertical pair):
            # one 1KB gather element covers the whole 2x2 bilinear quad.
            x_cp = dpool.tile([NPADR, 2 * P], f16, name="x_cp")
            w_rows = dpool.tile([36, NPOS], f16, name="w_rows")
            idx_rows = dpool.tile([K, NPOS], i16, name="idx_rows")
            # ---------- constants ----------
            ident16 = cpool.tile([P, P], f16)
            ident32 = cpool.tile([P, P], f32)
            ones1 = cpool.tile([1, P], f16)
            nc.vector.memset(ones1[:], 1.0)
            ones3 = cpool.tile([65, P], f16)
            nc.vector.memset(ones3[:], 1.0)
            onesP = cpool.tile([P, P], f32)
            nc.vector.memset(onesP[:], 1.0)
            ramp128 = cpool.tile([P, P], f32)
            nc.vector.tensor_tensor_scan(ramp128[:], onesP[:], onesP[:], -1.0,
                                         Alu.mult, Alu.add)
            pcol_d = dpool.tile([1, P], f32, name="pcol_d")
            nc.sync.dma_start(pcol_d[:], ramp128[0:1, :])
            pcol = cpool.tile([P, 1], f32)
            src_p = AP(tensor=_h(pcol_d), offset=0, ap=[[1, P], [1, 1]])
            nc.sync.dma_start(pcol[:], src_p)
            nc.vector.tensor_scalar(ident32[:], ramp128[:], pcol[:], None,
                                    Alu.is_equal)
            nc.vector.tensor_copy(ident16[:], ident32[:])
            # wrap-permutation matrix: Pprm[p, f] = 1 iff p = (f%8)*16 + f//8.
            # Used for the index transposes so idx stores land wrap-major.
            prow = cpool.tile([1, P], f32)
            prowi = cpool.tile([1, P], i32)
            nc.vector.tensor_scalar(prow[:], ramp128[0:1, :], 1.0 / 16.0,
                                    None, Alu.mult)
            nc.vector.tensor_copy(prowi[:], prow[:])
            nc.vector.tensor_copy(prow[:], prowi[:])          # q16 = c//16
            ppd = dpool.tile([1, P], f32, name="ppd")
            prow2 = cpool.tile([1, P], f32)
            # val = 8*(c - 16*q16) + q16 = 8*c - 128*q16 + q16
            nc.vector.tensor_scalar(prow2[:], prow[:], -127.0, None, Alu.mult)
            nc.vector.scalar_tensor_tensor(prow2[:], ramp128[0:1, :], 8.0,
                                           prow2[:], Alu.mult, Alu.add)
            nc.sync.dma_start(ppd[:], prow2[:])
            pcolI = cpool.tile([P, 1], f32)
            src_pi = AP(tensor=_h(ppd), offset=0, ap=[[1, P], [1, 1]])
            nc.sync.dma_start(pcolI[:], src_pi)
            Pprm32 = cpool.tile([P, P], f32)
            nc.vector.tensor_scalar(Pprm32[:], ramp128[:], pcolI[:], None,
                                    Alu.is_equal)

            # ---------- persistent tiles ----------
            idxw = ppool.tile([P, K * 576], i16)
            WkT = ppool.tile([P, K * P], f16)

            with tc.tile_pool(name="prepA", bufs=2) as pa:
                x16p = pa.tile([P, NPADR], f16, tag="x16p")
                w16 = pa.tile([P, 1152], f16, tag="w16")
                offt = pa.tile([P, NT * 18], f32, tag="offt")
                # ---- loads first (in-order DMA queue) ----
                with tc.tile_pool(name="ld", bufs=1) as pld, \
                     tc.tile_pool(name="psoP", bufs=2, space="PSUM") as psoP:
                    x_sb = pld.tile([P, NPOS], f32, tag="xsb")
                    nc.sync.dma_start(x_sb[:, 0:NPOS // 2],
                                      x_in[:, 0:NPOS // 2])
                    nc.sync.dma_start(x_sb[:, NPOS // 2:],
                                      x_in[:, NPOS // 2:])
                    off_sb = pld.tile([18, NPOS], f32, tag="offsb")
                    nc.sync.dma_start(off_sb[:], off_in[:])
                    w_sb = pld.tile([P, 1152], f32, tag="wsb")
                    nc.sync.dma_start(w_sb[:], w_in[:])

                    nc.gpsimd.memset(x16p[:], 0.0)
                    # interior: x16p[:, (y+1)*98 + (x+1)] = f16(x[:, y*96+x])
                    dst = x16p[:, PW:PW + H * PW].rearrange(
                        "p (r w) -> p r w", w=PW)[:, :, 1:1 + W]
                    srcx = x_sb[:].rearrange("p (r w) -> p r w", w=W)
                    nc.scalar.copy(dst[:, 0:H // 2, :], srcx[:, 0:H // 2, :])
                    nc.vector.tensor_copy(dst[:, H // 2:, :],
                                          srcx[:, H // 2:, :])

                    for tg in range(3):
                        pso = psoP.tile([P, 24 * 18], f32, tag="pso")
                        for j in range(24):
                            t = tg * 24 + j
                            nc.tensor.transpose(pso[:, j * 18:(j + 1) * 18],
                                                off_sb[0:18, t * P:(t + 1) * P],
                                                ident32[0:18, 0:18])
                        nc.scalar.copy(offt[:, tg * 432:(tg + 1) * 432], pso[:])

                    nc.scalar.copy(w16[:], w_sb[:])

                # ---- phase A: transpose padded image to pixel-major ----
                # Each pixel row j is stored twice: as the top half of pair
                # j and the bottom half of pair j-98.
                with tc.tile_pool(name="ptP", bufs=2, space="PSUM") as ptP:
                    groups = [8] * 9 + [4]
                    t0 = 0
                    for gi, gw in enumerate(groups):
                        pt8 = ptP.tile([P, 8 * P], f16, tag="pt8")
                        for j in range(gw):
                            t = t0 + j
                            nc.tensor.transpose(pt8[:, j * P:(j + 1) * P],
                                                x16p[:, t * P:(t + 1) * P],
                                                ident16[:])
                        ev = pa.tile([P, 8 * P], f16, tag="ev")
                        if gi % 2 == 1:
                            nc.vector.tensor_copy(ev[:, 0:gw * P],
                                                  pt8[:, 0:gw * P])
                        else:
                            nc.scalar.copy(ev[:, 0:gw * P], pt8[:, 0:gw * P])
                        dst1 = AP(tensor=_h(x_cp), offset=t0 * P * 2 * P,
                                  ap=[[2 * P, P], [128 * 2 * P, gw], [1, P]])
                        nc.sync.dma_start(
                            dst1,
                            ev[:, 0:gw * P].rearrange("r (j c) -> r j c",
                                                      j=gw))
                        if gi == 0:
                            dst2a = AP(tensor=_h(x_cp), offset=P,
                                       ap=[[2 * P, 30], [1, P]])
                            nc.sync.dma_start(dst2a, ev[98:128, 0:P])
                            dst2b = AP(tensor=_h(x_cp),
                                       offset=30 * 2 * P + P,
                                       ap=[[2 * P, P], [128 * 2 * P, gw - 1],
                                           [1, P]])
                            nc.sync.dma_start(
                                dst2b,
                                ev[:, P:gw * P].rearrange(
                                    "r (j c) -> r j c", j=gw - 1))
                        else:
                            dst2 = AP(tensor=_h(x_cp),
                                      offset=(t0 * P - 98) * 2 * P + P,
                                      ap=[[2 * P, P], [128 * 2 * P, gw],
                                          [1, P]])
                            nc.sync.dma_start(
                                dst2,
                                ev[:, 0:gw * P].rearrange("r (j c) -> r j c",
                                                          j=gw))
                        t0 += gw
                    # zero the never-paired tail bottom halves (pairs
                    # 9630..9727) so the gather window view is fully defined
                    ztail = AP(tensor=_h(x_cp),
                               offset=(NPADR - 98) * 2 * P + P,
                               ap=[[2 * P, 98], [1, P]])
                    nc.sync.dma_start(ztail, x16p[0:98, 9728 - P:9728])

                    # ---- conv weights -> WkT ----
                    for k in range(K):
                        wkc = pa.tile([P, P], f16, tag="wkc")
                        nc.scalar.copy(wkc[:], w16[:, k:1152:9])
                        ptw = ptP.tile([P, P], f16, tag="ptw")
                        nc.tensor.transpose(ptw[:], wkc[:], ident16[:])
                        nc.scalar.copy(WkT[:, k * P:(k + 1) * P], ptw[:])

                # ---- phase C: position/weight/index math (DVE) ----
                with tc.tile_pool(name="pc", bufs=1) as pc:
                    def st(tag, dt=f32):
                        return pc.tile([P, NT], dt, tag=tag, name=tag)

                    def mt(tag, dt=f32):
                        return pc.tile([P, NF], dt, tag=tag, name=tag)

                    posf = st("posf")
                    nc.vector.tensor_scalar(posf[:], ramp128[:, 0:NT], 128.0,
                                            None, Alu.mult)
                    nc.vector.tensor_scalar(posf[:], posf[:], pcol[:], None,
                                            Alu.add)
                    q0i = st("q0i", i32)
                    tmpq = st("tmpq")
                    nc.vector.tensor_scalar(tmpq[:], posf[:], 1.0 / 96.0, None,
                                            Alu.mult)
                    nc.vector.tensor_copy(q0i[:], tmpq[:])
                    q0 = st("q0")
                    nc.vector.tensor_copy(q0[:], q0i[:])
                    r0 = st("r0")
                    nc.vector.scalar_tensor_tensor(r0[:], q0[:], -96.0, posf[:],
                                                   Alu.mult, Alu.add)
                    ltz = st("ltz")
                    nc.vector.tensor_scalar(ltz[:], r0[:], 0.0, None, Alu.is_lt)
                    gez = st("gez")
                    nc.vector.tensor_scalar(gez[:], r0[:], 96.0, None, Alu.is_ge)
                    Rr = st("Rr")
                    nc.vector.tensor_tensor(Rr[:], q0[:], ltz[:], Alu.subtract)
                    nc.vector.tensor_tensor(Rr[:], Rr[:], gez[:], Alu.add)
                    Cc = st("Cc")
                    nc.vector.scalar_tensor_tensor(Cc[:], ltz[:], 96.0, r0[:],
                                                   Alu.mult, Alu.add)
                    nc.vector.scalar_tensor_tensor(Cc[:], gez[:], -96.0, Cc[:],
                                                   Alu.mult, Alu.add)

                    T1 = mt("T1")
                    T2 = mt("T2")
                    T3 = mt("T3")
                    T4 = mt("T4")
                    T5 = mt("T5")
                    T6 = mt("T6")
                    T7 = mt("T7")
                    T8 = mt("T8")
                    VI = mt("VI", i32)
                    A0 = mt("A0", f16)
                    A1 = mt("A1", f16)
                    B0 = mt("B0", f16)
                    B1 = mt("B1", f16)

                    for k in range(K):
                        ky, kx = k // 3, k % 3
                        nc.vector.tensor_scalar(T1[:, k * NT:(k + 1) * NT],
                                                Rr[:], float(ky - 1), None,
                                                Alu.add)
                        nc.vector.tensor_scalar(T2[:, k * NT:(k + 1) * NT],
                                                Cc[:], float(kx - 1), None,
                                                Alu.add)
                    offv = offt[:].rearrange("p (t pl) -> p pl t", pl=18)
                    # py (T1), px (T2)
                    nc.vector.tensor_tensor(
                        T1[:].rearrange("p (k t) -> p k t", k=K),
                        offv[:, 0:18:2, :],
                        T1[:].rearrange("p (k t) -> p k t", k=K), Alu.add)
                    nc.vector.tensor_tensor(
                        T2[:].rearrange("p (k t) -> p k t", k=K),
                        offv[:, 1:18:2, :],
                        T2[:].rearrange("p (k t) -> p k t", k=K), Alu.add)

                    def floor_frac(v, vf, fr, ng):
                        nc.vector.tensor_copy(VI[:], v[:])
                        nc.vector.tensor_copy(vf[:], VI[:])
                        nc.vector.tensor_tensor(fr[:], v[:], vf[:], Alu.subtract)
                        nc.vector.tensor_scalar(ng[:], fr[:], 0.0, None,
                                                Alu.is_lt)
                        nc.vector.tensor_tensor(vf[:], vf[:], ng[:],
                                                Alu.subtract)
                        nc.vector.tensor_tensor(fr[:], fr[:], ng[:], Alu.add)

                    floor_frac(T1, T3, T4, T7)   # y0=T3, fy=T4
                    floor_frac(T2, T5, T6, T7)   # x0=T5, fx=T6

                    # mask mm = (y0 in [-1,95]) & (x0 in [-1,95]) -> T8
                    nc.vector.tensor_scalar(T8[:], T3[:], -1.0, None, Alu.is_ge)
                    nc.vector.tensor_scalar(T1[:], T3[:], 95.0, None, Alu.is_le)
                    nc.vector.tensor_tensor(T8[:], T8[:], T1[:], Alu.mult)
                    nc.vector.tensor_scalar(T1[:], T5[:], -1.0, None, Alu.is_ge)
                    nc.vector.tensor_scalar(T2[:], T5[:], 95.0, None, Alu.is_le)
                    nc.vector.tensor_tensor(T1[:], T1[:], T2[:], Alu.mult)
                    nc.vector.tensor_tensor(T8[:], T8[:], T1[:], Alu.mult)

                    # wbot (T2) = fy*mm ; wtop (T1) = mm - wbot ; omfx (T7)
                    nc.vector.tensor_tensor(T2[:], T4[:], T8[:], Alu.mult)
                    nc.vector.tensor_tensor(T1[:], T8[:], T2[:], Alu.subtract)
                    nc.vector.tensor_scalar(T7[:], T6[:], -1.0, 1.0, Alu.mult,
                                            Alu.add)
                    nc.vector.tensor_tensor(A0[:], T1[:], T7[:], Alu.mult)
                    nc.vector.tensor_tensor(A1[:], T1[:], T6[:], Alu.mult)
                    nc.vector.tensor_tensor(B0[:], T2[:], T7[:], Alu.mult)
                    nc.vector.tensor_tensor(B1[:], T2[:], T6[:], Alu.mult)

                    # jT = (clip(y0)+1)*98 + clip(x0)+1 ; jB = jT + 98
                    nc.vector.tensor_scalar(T3[:], T3[:], -1.0, 95.0, Alu.max,
                                            Alu.min)
                    nc.vector.tensor_scalar(T5[:], T5[:], -1.0, 95.0, Alu.max,
                                            Alu.min)
                    nc.vector.tensor_scalar(T5[:], T5[:], 99.0, None, Alu.add)
                    nc.vector.scalar_tensor_tensor(T4[:], T3[:], 98.0, T5[:],
                                                   Alu.mult, Alu.add)   # JT

                    # ---- phase D: to row-major DRAM ----
                    # weight row order matches gather slot order:
                    # (y0x0, y1x0, y0x1, y1x1) -> (A0, B0, A1, B1)
                    with tc.tile_pool(name="prepDp", bufs=2,
                                      space="PSUM") as pdp:
                        for k in range(K):
                            psw = pdp.tile([NT, 4 * P], f16, tag="psw")
                            for s, tt_ in enumerate((A0, B0, A1, B1)):
                                nc.tensor.transpose(psw[:, s * P:(s + 1) * P],
                                                    tt_[:, k * NT:(k + 1) * NT],
                                                    ident16[:])
                            evw = pa.tile([NT, 4 * P], f16, tag="evw")
                            nc.scalar.copy(evw[:], psw[:])
                            dstw = AP(tensor=_h(w_rows), offset=(4 * k) * NPOS,
                                      ap=[[P, NT], [NPOS, 4], [1, P]])
                            nc.sync.dma_start(
                                dstw, evw[:].rearrange("c (s e) -> c s e", s=4))

                            psi = pdp.tile([NT, P], f32, tag="psi")
                            nc.tensor.transpose(psi[:],
                                                T4[:, k * NT:(k + 1) * NT],
                                                ident32[:])
                            evi0 = pa.tile([NT, P], i16, tag="evi0")
                            nc.vector.tensor_copy(evi0[:], psi[:])
                            # pre-permute on DVE so the wrap-major store has
                            # contiguous runs: evi[c, el*8+eh] = evi0[c, e]
                            # with e = eh*16+el
                            evi = pa.tile([NT, P], i16, tag="evi")
                            nc.vector.tensor_copy(
                                evi[:].rearrange("c (el eh) -> c el eh",
                                                 el=16, eh=8),
                                evi0[:].rearrange("c (eh el) -> c el eh",
                                                  eh=8, el=16))
                            # idx i=t*128+eh*16+el sits at evi free f=el*8+eh;
                            # store to w = el*576 + 8t + eh (wrap-major)
                            dsti = AP(tensor=_h(idx_rows),
                                      offset=k * NPOS,
                                      ap=[[8, NT], [576, 16], [1, 8]])
                            srci = evi[:].rearrange(
                                "c (el eh) -> c el eh", el=16, eh=8)
                            nc.sync.dma_start(dsti, srci)

                            # phase E (per k): wrapped load; k=0 replicated
                            # eagerly so the first gather starts early, the
                            # rest in three wide DMAs after the last tap.
                            srcq = AP(tensor=_h(idx_rows), offset=k * NPOS,
                                      ap=[[576, 16], [1, 576]])
                            eng = nc.gpsimd if k == 0 else nc.sync
                            eng.dma_start(
                                idxw[0:16, k * 576:(k + 1) * 576], srcq)
                            if k == 0:
                                ks = slice(0, 576)
                                nc.sync.dma_start(idxw[16:32, ks],
                                                  idxw[0:16, ks])
                                nc.sync.dma_start(idxw[32:64, ks],
                                                  idxw[0:32, ks])
                                nc.sync.dma_start(idxw[64:128, ks],
                                                  idxw[0:64, ks])
                        ks = slice(576, K * 576)
                        nc.sync.dma_start(idxw[16:32, ks], idxw[0:16, ks])
                        nc.sync.dma_start(idxw[32:64, ks], idxw[0:32, ks])
                        nc.sync.dma_start(idxw[64:128, ks], idxw[0:64, ks])

            # ---------- phase F: main loop ----------
            xt_win = AP(tensor=_h(x_cp), offset=0,
                        ap=[[2 * P, NPADR - 1], [1, 4 * P]])
            evac_ctr = 0
            with tc.tile_pool(name="g", bufs=int(os.environ.get("GB", "3"))) as gp, \
                 tc.tile_pool(name="wstp", bufs=2) as wstp, \
                 tc.tile_pool(name="wstPp", bufs=3) as wstPp, \
                 tc.tile_pool(name="aw16p", bufs=int(os.environ.get("AB", "2"))) as aw16p, \
                 tc.tile_pool(name="c4p", bufs=int(os.environ.get("CB", "2"))) as c4p, \
                 tc.tile_pool(name="osp", bufs=2) as osp, \
                 tc.tile_pool(name="awps", bufs=2, space="PSUM") as awps, \
                 tc.tile_pool(name="outps", bufs=1, space="PSUM") as outps:
                def iter_seq():
                    for c in range(NCH):
                        for k in range(K):
                            yield c, k

                GW = 768  # per-gather index count (hw limit <= 896)

                def issue_gather(c, k):
                    g4 = gp.tile([P, CW // GW, 4, GW], f16, tag="g4",
                                 name="g4")
                    for hh in range(CW // GW):
                        i0 = k * 576 + (c * CW + hh * GW) // 16
                        nc.gpsimd.dma_gather(
                            g4[:, hh], xt_win,
                            idxw[:, i0:i0 + GW // 16],
                            num_idxs=GW, num_idxs_reg=GW,
                            elem_size=4 * P, elem_step=2 * P, transpose=True)
                    return g4

                def issue_wst3(c, g):
                    # one DMA loads 3 taps' weight rows (12 rows) into a
                    # 3-partition tile; prefetched a group ahead so the small
                    # transfer isn't starved behind gathers on the DMA track
                    w3 = wstp.tile([65, 4, CW], f16, tag="wst3")
                    wsrc = AP(tensor=_h(w_rows),
                              offset=(4 * 3 * g) * NPOS + c * CW,
                              ap=[[4 * NPOS, 3], [NPOS, 4], [1, CW]])
                    nc.sync.dma_start(w3[0:65:32, :, :], wsrc)
                    return w3

                iters = list(iter_seq())
                wgroups = [(c, g) for c in range(NCH) for g in range(3)]
                pending_w = [issue_wst3(*wgroups[0])]
                wg_idx = 0

                uo_list = os.environ.get("UO", "ADAAPA").split("/")

                def p_unit_of(it):
                    order = uo_list[it % len(uo_list)]
                    for ui, u in enumerate(order):
                        if u == 'P':
                            return ui // 3, ui % 3
                    return None

                def issue_wstP(it):
                    pu = p_unit_of(it)
                    if pu is None:
                        return None
                    c, k = iters[it]
                    half, h = pu
                    wstP = wstPp.tile([1, 2, HB], f16, tag="wstP")
                    wpsrc = AP(tensor=_h(w_rows),
                               offset=(4 * k + 2 * half) * NPOS
                               + c * CW + h * HB,
                               ap=[[NPOS, 2], [1, HB]])
                    nc.sync.dma_start(wstP[:], wpsrc.unsqueeze(0))
                    return wstP

                pending_p = [issue_wstP(0), issue_wstP(1)]
                g4_next = issue_gather(*iters[0])
                for idx_it, (c, k) in enumerate(iters):
                    if k == 0:
                        out_ps = outps.tile([P, CW], f32, tag="ops",
                                            name="out_ps")
                    g4 = g4_next
                    if idx_it + 1 < len(iters):
                        g4_next = issue_gather(*iters[idx_it + 1])
                    if k % 3 == 0:
                        wst3 = pending_w.pop(0)
                        wg_idx += 1
                        if wg_idx < len(wgroups):
                            pending_w.append(issue_wst3(*wgroups[wg_idx]))
                    kk = k % 3
                    wstP_cur = pending_p.pop(0)
                    if idx_it + 2 < len(iters):
                        pending_p.append(issue_wstP(idx_it + 2))
                    if True:

                        # per half (top slots 0:2 / bottom 2:4): 3 aw units
                        # per half as separate contiguous [P,2,HB] tiles; the
                        # 'P' unit is a Pool partition_broadcast from a small
                        # partition-0 staging tile (HW requires contiguous
                        # partition-0 source).
                        unit_order = uo_list[idx_it % len(uo_list)]
                        awU = [[None] * (CW // HB) for _ in range(2)]
                        for half in (0, 1):
                            for h in range(CW // HB):
                                aw1 = aw16p.tile([P, 2, HB], f16,
                                                 tag=f"aw{half}{h}")
                                awU[half][h] = aw1
                                u = unit_order[evac_ctr % 6]
                                evac_ctr += 1
                                if u == 'P':
                                    nc.gpsimd.partition_broadcast(
                                        aw1[:], wstP_cur[:])
                                    continue
                                awp_t = awps.tile([P, 2, HB], f32, tag="awp")
                                for s2 in range(2):
                                    nc.tensor.matmul(
                                        awp_t[:, s2, :],
                                        ones3[32 * kk:32 * kk + 1, :],
                                        wst3[32 * kk:32 * kk + 1,
                                             2 * half + s2,
                                             h * HB:(h + 1) * HB],
                                        start=True, stop=True)
                                if u == 'D':
                                    nc.vector.tensor_copy(aw1[:], awp_t[:])
                                else:
                                    nc.scalar.copy(aw1[:], awp_t[:])

                        for half in (0, 1):
                            c4 = c4p.tile([P, 2, CW], f16, tag=f"c4{half}")
                            # weight segments: intersections of the 512-wide
                            # aw tiles with the 768-wide gather sub-blocks
                            for h in range(CW // HB):
                                p0 = h * HB
                                while p0 < (h + 1) * HB:
                                    hh = p0 // GW
                                    p1 = min((h + 1) * HB, (hh + 1) * GW)
                                    nc.vector.tensor_tensor(
                                        c4[:, :, p0:p1],
                                        g4[:, hh, 2 * half:2 * half + 2,
                                           p0 - hh * GW:p1 - hh * GW],
                                        awU[half][h][:, :,
                                                     p0 - h * HB:p1 - h * HB],
                                        Alu.mult)
                                    p0 = p1
                            for s2 in range(2):
                                for j in range(CW // HB):
                                    nc.tensor.matmul(
                                        out_ps[:, j * HB:(j + 1) * HB],
                                        WkT[:, k * P:(k + 1) * P],
                                        c4[:, s2, j * HB:(j + 1) * HB],
                                        start=(k == 0 and half == 0
                                               and s2 == 0),
                                        stop=(k == K - 1 and half == 1
                                              and s2 == 1),
                                        skip_group_check=True)
                    if k == K - 1:
                        osb = osp.tile([P, CW], f32, tag="osb")
                        nc.scalar.copy(osb[:], out_ps[:])
                        nc.sync.dma_start(out[:, c * CW:(c + 1) * CW], osb[:])
    nc.compile()
    return nc


_NC = None


def kernel(x, offset, weight):
    global _NC
    if _NC is None:
        _NC = build_nc()
    from concourse.bass_utils import run_bass_kernel_spmd
    B = x.shape[0]
    w2 = np.ascontiguousarray(np.asarray(weight).reshape(P, 1152)).astype(np.float32)
    in_maps = []
    for b in range(B):
        in_maps.append({
            "x": np.ascontiguousarray(np.asarray(x)[b].reshape(P, NPOS), dtype=np.float32),
            "offset": np.ascontiguousarray(np.asarray(offset)[b].reshape(18, NPOS), dtype=np.float32),
            "weight": w2,
        })
    res = run_bass_kernel_spmd(_NC, in_maps, list(range(B)))
    outs = [res.results[b]["out"].reshape(P, H, W) for b in range(B)]
    return np.stack(outs).astype(np.float32)
